# revision 28
# baseline (speedup 1.0000x reference)
"""Bass/Tile SPMD kernel for GQA attention prefill (B=2,S=2048,D=4096,H=32,KVH=8,HD=128).

Sharding: 8 cores = 2 batch-groups x 4 cores. Core r of a batch group owns
four 128-row q-tiles, one per causal-need quartile: [12+r | 8+r | 4+r | r]
(SBUF column slots A..D). Total causal work is uniform across cores and the
SPMD instruction graph is identical; per-core causality comes from mask DATA.

Per core:
  phase 1: K^T projection (ch-outer so the first matmul starts ~1.3MB into
           the x DMA) + RoPE -> DRAM bounce -> AllGather over the 4-core
           batch group; V projection; Q projection + RoPE (first 6 ets'
           weights prefetched ahead of the wv stream; the V AllGather is
           deferred into the Q window to spread HBM demand; kh=0's K/V
           gather is prefetched on the GpSimd queue behind the collectives).
  phase 2: attention, [k,q] orientation. Step kt (0..15) covers cols [0:w],
           w = 512-128*(kt//4); the one slot whose diagonal can fall in kt
           is always cols [w-128:w], masked with per-core data. Scores are
           emitted 3 steps ahead of PV so the in-order PE never waits on the
           ScalarE exp. Rowsum: P-tile pairs pre-added on Vector, then a
           [128,128]-ones matmul (half the moving rows; result lands
           pre-broadcast on all 128 partitions), fast-approx reciprocal +
           multiply on Vector.
  phase 3: out rows = attn_T @ wo with the full wo streamed once.
           Output rows are disjoint across cores -> no reduction collective.

All data bf16 (1 PE cycle/row); PSUM accumulation f32. Measured PE rate is
~1.95GHz (board power throttle caps utilization at 0.8125 of 2.4GHz).
"""
import numpy as np
import concourse.bass as bass
import concourse.mybir as mybir
import concourse.tile as tile
from concourse import bacc

F32 = mybir.dt.float32
R32 = mybir.dt.float32r
BF16 = mybir.dt.bfloat16

B, S, D = 2, 2048, 4096
H, KVH, HD = 32, 8, 128
NBLK, BLK = 8, 256          # causal query blocks
SW = 512                    # stripe rows per core (2 blocks)
NDT = D // 128              # 32 contraction tiles
NET_Q = (H * HD) // 128     # 32
NET_KV = (KVH * HD) // 128  # 8
NKT0, NKT1 = 8, 16          # uniform k-tile counts (low block, high block)
SCALE = float(1.0 / np.sqrt(HD))
KV_REGION = 1024 * 512      # elems: K slab [1024,512] / V slab [512,1024]

REPLICA_GROUPS = [[0, 1, 2, 3], [4, 5, 6, 7]]


def _r(ap):
    return ap


def build():
    nc = bacc.Bacc("TRN2", target_bir_lowering=False, debug=False, num_devices=8)

    xt = nc.declare_dram_parameter("xt", [D, SW], BF16, isOutput=False)
    wqt = nc.declare_dram_parameter("wqt", [NET_Q, 4, 128, 8, 128], BF16, isOutput=False)
    wkt = nc.declare_dram_parameter("wkt", [NET_KV, 4, 128, 8, 128], BF16, isOutput=False)
    wvt = nc.declare_dram_parameter("wvt", [D, KVH * HD], BF16, isOutput=False)
    wot = nc.declare_dram_parameter("wot", [H * HD, D], BF16, isOutput=False)
    cos2 = nc.declare_dram_parameter("cos2", [128, SW], BF16, isOutput=False)
    sin2 = nc.declare_dram_parameter("sin2", [128, SW], BF16, isOutput=False)
    mq = nc.declare_dram_parameter("mq", [128, 16 * 128], BF16, isOutput=False)
    out = nc.declare_dram_parameter("out", [SW, D], F32, isOutput=True)

    with tile.TileContext(nc) as tc:
        _body(nc, tc, xt.ap(), wqt.ap(), wkt.ap(), wvt.ap(), wot.ap(),
              cos2.ap(), sin2.ap(), mq.ap(), out.ap())

    nc.compile()
    return nc


def _body(nc, tc, xt, wqt, wkt, wvt, wot, cos2, sin2, mq, out):
    from contextlib import ExitStack

    es = ExitStack()
    with es:
        # ---- persistent pools (live through phases 1+2) ----
        const_pool = es.enter_context(tc.tile_pool(name="consts", bufs=1))
        qkv_pool = es.enter_context(tc.tile_pool(name="qkv", bufs=1))
        dram = es.enter_context(tc.tile_pool(name="dram", bufs=1, space="DRAM"))

        cos_sb = const_pool.tile([128, SW], BF16)
        sin_sb = const_pool.tile([128, SW], BF16)
        mq_sb = const_pool.tile([128, 16, 128], BF16)
        ones_sb = const_pool.tile([128, 128], BF16)
        # consts ride the (otherwise idle) Scalar engine's DMA trigger queue
        # so the Sync queue starts on x/weight chunks immediately.
        nc.scalar.dma_start(cos_sb[:], cos2)
        nc.scalar.dma_start(sin_sb[:], sin2)
        nc.scalar.dma_start(mq_sb[:], mq.rearrange("p (t c) -> p t c", c=128))
        nc.vector.memset(ones_sb[:], 1.0)

        q_sb = qkv_pool.tile([128, NET_Q, SW], BF16)      # Q_T, head-major
        k_in = dram.tile([KV_REGION], BF16)
        v_in = dram.tile([KV_REGION], BF16)
        k_out = dram.tile([4, KV_REGION], BF16)
        v_out = dram.tile([4, KV_REGION], BF16)
        kreg = [k_out[gs].rearrange("(et p c) -> p et c", et=NET_KV, p=128)
                for gs in range(4)]
        vreg = [v_out[gs].rearrange("(st p e) -> p st e", st=4, p=128)
                for gs in range(4)]
        kvload = es.enter_context(tc.tile_pool(name="p2kv", bufs=3))

        def load_kv(kh, dma_start):
            k_kh = kvload.tile([128, NKT1 * 128], BF16, tag="k_kh")
            v_kh = kvload.tile([128, NKT1, 128], BF16, tag="v_kh")
            for t in range(16):
                gsrc = t % 4
                slot = 3 - t // 4
                dma_start(
                    k_kh[:, t * 128:(t + 1) * 128],
                    kreg[gsrc][:, kh, slot * 128:(slot + 1) * 128])
                dma_start(
                    v_kh[:, t, :],
                    vreg[gsrc][:, slot, kh * 128:(kh + 1) * 128])
            return k_kh, v_kh

        # ================= phase 1: projections =================
        with tc.tile_pool(name="p1x", bufs=1) as xpool, \
             tc.tile_pool(name="p1w", bufs=24) as wpool, \
             tc.tile_pool(name="p1wv", bufs=8) as wvpool, \
             tc.tile_pool(name="p1kv", bufs=1) as kvpool, \
             tc.tile_pool(name="p1rope", bufs=2) as rpool:

            x_sb = xpool.tile([128, NDT, SW], BF16)
            xtr = xt.rearrange("(dt p) c -> p dt c", p=128)

            k_sb = kvpool.tile([128, NET_KV, SW], BF16)
            v_sb = kvpool.tile([128, SW // 128, KVH * HD], BF16)

            def rope(ps, dst):
                raw = rpool.tile([128, SW], BF16, tag="rope_raw")
                sw_t = rpool.tile([128, SW], BF16, tag="rope_sw")
                nc.vector.tensor_copy(raw[:], ps)
                nc.sync.dma_start(sw_t[0:64, :], raw[64:128, :])
                nc.sync.dma_start(sw_t[64:128, :], raw[0:64, :])
                nc.vector.tensor_mul(dst, ps, cos_sb[:])
                nc.vector.tensor_mul(sw_t[:], sw_t[:], sin_sb[:])
                nc.vector.tensor_add(dst, dst, sw_t[:])

            # K projection + RoPE (first: feeds the AllGather). ch-outer
            # order: each ch needs only x chunk ch, so the first matmul
            # starts after ~1.3 MB of DMA instead of the full 4.2 MB x.
            with tc.tile_pool(name="p1kps", bufs=1, space="PSUM") as kpspool:
                kps = [kpspool.tile([128, SW], F32, tag=f"kps{et}",
                                    name=f"kps{et}")
                       for et in range(NET_KV)]
                for ch in range(4):
                    nc.sync.dma_start(
                        x_sb[:, ch * 8:(ch + 1) * 8],
                        xtr[:, ch * 8:(ch + 1) * 8])
                    for et in range(NET_KV):
                        w = wpool.tile([128, 8, 128], BF16, tag="wchunk")
                        nc.sync.dma_start(w[:], wkt[et, ch])
                        for dl in range(8):
                            dt = ch * 8 + dl
                            nc.tensor.matmul(
                                kps[et][:], _r(w[:, dl]), _r(x_sb[:, dt]),
                                start=(dt == 0), stop=(dt == NDT - 1))
                for et in range(NET_KV):
                    rope(kps[et][:], k_sb[:, et])
            nc.gpsimd.dma_start(
                k_in.rearrange("(et p c) -> p et c", et=NET_KV, p=128),
                k_sb[:])
            nc.gpsimd.collective_compute(
                "AllGather", mybir.AluOpType.bypass,
                replica_groups=REPLICA_GROUPS,
                ins=[k_in.opt()], outs=[k_out.opt()])

            with tc.tile_pool(name="p1ps", bufs=2, space="PSUM") as pspool, \
                 tc.tile_pool(name="p1psv", bufs=1, space="PSUM") as psvpool:

                def project(dst_slice, w_dram, et, pre=None):
                    # dst_slice: [128, SW] target; accumulate over 32 d-tiles
                    ps = pspool.tile([128, SW], F32, tag="proj_ps")
                    for ch in range(4):  # 8 d-tiles per weight chunk
                        if pre is not None:
                            w = pre[ch]
                        else:
                            w = wpool.tile([128, 8, 128], BF16, tag="wchunk")
                            nc.sync.dma_start(w[:], w_dram[et, ch])
                        for dl in range(8):
                            dt = ch * 8 + dl
                            nc.tensor.matmul(
                                ps[:], _r(w[:, dl]), _r(x_sb[:, dt]),
                                start=(dt == 0), stop=(dt == NDT - 1))
                    rope(ps[:], dst_slice)

                # prefetch the first 6 Q-weight ets on the Sync queue AHEAD
                # of the wv stream: they land by ~t=60us, so the Q projection
                # starts hot even while AG_V still owns HBM bandwidth.
                preq = []
                for et in range(6):
                    chunks = []
                    for ch in range(4):
                        w = wpool.tile([128, 8, 128], BF16, tag="wchunk")
                        nc.sync.dma_start(w[:], wqt[et, ch])
                        chunks.append(w)
                    preq.append(chunks)

                # V projection (natural [s, e] orientation; x as stationary)
                for ech in range(2):
                    vps = [psvpool.tile([128, 512], F32, tag=f"vps{st}",
                                        name=f"vps{st}")
                           for st in range(4)]
                    for dt in range(NDT):
                        wv = wvpool.tile([128, 512], BF16, tag="wvchunk")
                        nc.sync.dma_start(
                            wv[:],
                            wvt.rearrange("(dt p) e -> p dt e", p=128)[
                                :, dt, ech * 512:(ech + 1) * 512])
                        for st in range(4):
                            nc.tensor.matmul(
                                vps[st][:],
                                _r(x_sb[:, dt, st * 128:(st + 1) * 128]),
                                _r(wv[:]),
                                start=(dt == 0), stop=(dt == NDT - 1))
                    for st in range(4):
                        nc.vector.tensor_copy(
                            v_sb[:, st, ech * 512:(ech + 1) * 512], vps[st][:])

                # Q projection + RoPE. The V bounce + AllGather are deferred
                # until Q is underway: the V-era HBM window is oversubscribed
                # (wv stream + AG_K already run there), while the Q window has
                # bandwidth to spare. kh=0's K/V gather is prefetched on the
                # GpSimd queue, which also hosts the collectives, so the
                # triggers are naturally ordered after AG completion.
                kv0 = None
                for et in range(NET_Q):
                    project(q_sb[:, et], wqt, et,
                            pre=preq[et] if et < len(preq) else None)
                    if et == 7:
                        nc.gpsimd.dma_start(
                            v_in.rearrange("(st p e) -> p st e", st=4, p=128),
                            v_sb[:])
                        nc.gpsimd.collective_compute(
                            "AllGather", mybir.AluOpType.bypass,
                            replica_groups=REPLICA_GROUPS,
                            ins=[v_in.opt()], outs=[v_out.opt()])
                        kv0 = load_kv(0, nc.gpsimd.dma_start)

        # ================= phase 2: attention =================
        attn_pool = es.enter_context(tc.tile_pool(name="attn", bufs=1))
        attn_sb = attn_pool.tile([128, NET_Q, SW], BF16)  # attn_T, head-major

        with tc.tile_pool(name="p2p", bufs=6) as ppool, \
             tc.tile_pool(name="p2n", bufs=2) as npool, \
             tc.tile_pool(name="p2ps_s", bufs=4, space="PSUM") as ps_s, \
             tc.tile_pool(name="p2ps_o", bufs=2, space="PSUM") as ps_o, \
             tc.tile_pool(name="p2ps_r", bufs=2, space="PSUM") as ps_r:

            # 4-slot exact-causal structure. Core r of each batch group owns
            # q-tiles (128 rows) [12+r | 8+r | 4+r | r] in cols [A|B|C|D].
            # Global k-tile t lives on group core t%4 at slot 3-t//4.
            # Step kt covers cols [0:w], w = 512-128*(kt//4); the slot whose
            # diagonal can fall in kt is always cols [w-128:w], masked with
            # per-core data (ones / triangle / zeros).
            for kh in range(KVH):
                k_kh, v_kh = kv0 if kh == 0 else load_kv(kh, nc.sync.dma_start)

                for qh4 in range(4):
                    h = kh * 4 + qh4
                    ops = ps_o.tile([128, 2 * BLK], F32, tag="ops")
                    rps = ps_r.tile([128, 2 * BLK], F32, tag="rps")

                    NSTEP = 16
                    sps_t = [None] * NSTEP
                    p_t = [None] * NSTEP
                    WID = [512 - 128 * (kt // 4) for kt in range(NSTEP)]

                    def emit_scores(i):
                        w = WID[i]
                        sps = ps_s.tile([128, 2 * BLK], F32, tag="sps")
                        sps_t[i] = sps
                        nc.tensor.matmul(
                            sps[:, 0:w], k_kh[:, i * 128:(i + 1) * 128],
                            q_sb[:, h, 0:w], start=True, stop=True)

                    def emit_expmask(i):
                        w = WID[i]
                        p_sb = ppool.tile([128, 2 * BLK], BF16, tag="p_sb")
                        p_t[i] = p_sb
                        nc.scalar.activation(
                            p_sb[:, 0:w], sps_t[i][:, 0:w],
                            mybir.ActivationFunctionType.Exp, scale=SCALE)
                        nc.vector.tensor_mul(
                            p_sb[:, w - 128:w], p_sb[:, w - 128:w],
                            mq_sb[:, i])

                    def emit_pv(i):
                        w = WID[i]
                        nc.tensor.matmul(
                            ops[:, 0:w], v_kh[:, i], p_t[i][:, 0:w],
                            start=(i == 0), stop=(i == NSTEP - 1),
                            skip_group_check=True)

                    pair_t = [None] * 8

                    def emit_rs_add(j):
                        # rowsum for step pair (2j, 2j+1): pre-add the two P
                        # tiles on Vector (same width within a quartile) so
                        # the ones-matmul runs half the moving rows.
                        w = WID[2 * j]
                        p_pair = ppool.tile([128, 2 * BLK], BF16,
                                            tag="p_pair")
                        pair_t[j] = p_pair
                        nc.vector.tensor_add(
                            p_pair[:, 0:w], p_t[2 * j][:, 0:w],
                            p_t[2 * j + 1][:, 0:w])

                    def emit_rs_mm(j):
                        w = WID[2 * j]
                        nc.tensor.matmul(
                            rps[:, 0:w], ones_sb[:], pair_t[j][:, 0:w],
                            start=(j == 0), stop=(j == 7),
                            skip_group_check=True)

                    emit_scores(0)
                    emit_scores(1)
                    emit_scores(2)
                    for i in range(NSTEP):
                        emit_expmask(i)
                        if i % 2 == 1:
                            emit_rs_add(i // 2)
                        if i + 3 < NSTEP:
                            emit_scores(i + 3)
                        emit_pv(i)
                        if i >= 2 and i % 2 == 0:
                            emit_rs_mm(i // 2 - 1)
                    emit_rs_mm(7)

                    # normalize: rps rows are all identical (ones stationary
                    # is [128,128]) so no partition broadcast is needed.
                    rcpb = npool.tile([128, 2 * BLK], F32, tag="rcpb")
                    nc.vector.reciprocal_approx_fast(rcpb[:], rps[:])
                    nc.vector.tensor_mul(attn_sb[:, h, :], ops[:], rcpb[:])

        # ================= phase 3: output projection =================
        with tc.tile_pool(name="p3w", bufs=16) as wopool, \
             tc.tile_pool(name="p3y", bufs=4) as ypool, \
             tc.tile_pool(name="p3ps", bufs=1, space="PSUM") as ps_y:

            wotr = wot.rearrange("(et p) d -> p et d", p=128)
            for dch in range(8):
                yps = [ps_y.tile([128, 512], F32, tag=f"yps{st}", name=f"yps{st}")
                       for st in range(4)]
                for et in range(NET_Q):
                    w = wopool.tile([128, 512], BF16, tag="wo_chunk")
                    nc.sync.dma_start(
                        w[:], wotr[:, et, dch * 512:(dch + 1) * 512])
                    for st in range(4):
                        nc.tensor.matmul(
                            yps[st][:],
                            _r(attn_sb[:, et, st * 128:(st + 1) * 128]),
                            _r(w[:]),
                            start=(et == 0), stop=(et == NET_Q - 1))
                for st in range(4):
                    y = ypool.tile([128, 512], F32, tag="y_sb")
                    nc.vector.tensor_copy(y[:], yps[st][:])
                    nc.sync.dma_start(
                        out[st * 128:(st + 1) * 128, dch * 512:(dch + 1) * 512],
                        y[:])


# ======================= host side =======================

def _perm_idx(nheads):
    """Within each 128-dim head block: evens then odds."""
    idx = []
    for hh in range(nheads):
        base = hh * HD
        idx.extend(base + j for j in range(0, HD, 2))
        idx.extend(base + j for j in range(1, HD, 2))
    return np.array(idx)


def host_prep(x_norm, wq, wk, wv, wo, freqs_cos, freqs_sin, mask):
    """Build the 8 per-core input maps."""
    import ml_dtypes
    bf16 = ml_dtypes.bfloat16
    f32 = np.float32
    x_norm = np.ascontiguousarray(x_norm, dtype=f32)
    wq = np.asarray(wq, dtype=f32)
    wk = np.asarray(wk, dtype=f32)
    wv = np.asarray(wv, dtype=f32)
    wo = np.asarray(wo, dtype=f32)
    fc = np.asarray(freqs_cos, dtype=f32)
    fs = np.asarray(freqs_sin, dtype=f32)
    mask = np.asarray(mask, dtype=f32)

    pq = _perm_idx(H)
    pk = _perm_idx(KVH)
    def pack_w(wt):
        # wt: [D, E] (d-major) -> [E//128, 4, 128, 8, 128] matching SBUF chunks
        E = wt.shape[1]
        p_ = wt.reshape(4, 8, 128, E // 128, 128)     # [ch, dl, p, et, e]
        return np.ascontiguousarray(p_.transpose(3, 0, 2, 1, 4))

    wqt = pack_w(wq[pq, :].T)                        # [32, 4, 128, 8, 128]
    wkt = pack_w(wk[pk, :].T)                        # [8, 4, 128, 8, 128]
    wvt = np.ascontiguousarray(wv.T)                 # [D, 1024]
    wot = np.ascontiguousarray(wo.T)                 # [4096, D]

    cosT = fc.T                                      # [64, S]
    sinT = fs.T
    cos_full = np.concatenate([cosT, cosT], axis=0)  # [128, S]
    sin_full = np.concatenate([-sinT, sinT], axis=0)

    keep = (mask == 0.0)                             # [S, S] bool, k<=q

    in_maps = []
    for c in range(8):
        b, r = c // 4, c % 4
        tiles = [12 + r, 8 + r, 4 + r, r]            # slots A..D (128 q each)
        cols = np.concatenate(
            [np.arange(t * 128, (t + 1) * 128) for t in tiles])
        xt = np.ascontiguousarray(x_norm[b].T[:, cols])
        cos2 = np.ascontiguousarray(cos_full[:, cols])
        sin2 = np.ascontiguousarray(sin_full[:, cols])

        # mq[:, kt, :]: mask for the one slot whose diagonal can fall in
        # step kt — q-tile 4*(kt//4)+r at cols [w-128:w].
        m = np.zeros((128, 16, 128), dtype=f32)
        for kt in range(16):
            t_masked = 4 * (kt // 4) + r
            qrows = np.arange(t_masked * 128, (t_masked + 1) * 128)
            krows = np.arange(kt * 128, (kt + 1) * 128)
            m[:, kt, :] = keep[np.ix_(qrows, krows)].T

        in_maps.append({
            "xt": xt.astype(bf16),
            "wqt": wqt.astype(bf16), "wkt": wkt.astype(bf16),
            "wvt": wvt.astype(bf16), "wot": wot.astype(bf16),
            "cos2": cos2.astype(bf16), "sin2": sin2.astype(bf16),
            "mq": np.ascontiguousarray(m.reshape(128, 16 * 128)).astype(bf16),
        })
    return in_maps


def assemble(results):
    """results: list of 8 dicts with 'out' [512, 4096] -> full [B, S, D]."""
    full = np.empty((B, S, D), dtype=np.float32)
    for c in range(8):
        b, r = c // 4, c % 4
        tiles = [12 + r, 8 + r, 4 + r, r]
        rc = results[c]["out"]
        for slot, t in enumerate(tiles):
            full[b, t * 128:(t + 1) * 128] = rc[slot * 128:(slot + 1) * 128]
    return full


# ======================= public entry point =======================

_NC_CACHE = {}
last_exec_time_ns = None


def _get_nc():
    if "nc" not in _NC_CACHE:
        _NC_CACHE["nc"] = build()
    return _NC_CACHE["nc"]


def kernel(x_norm, wq, wk, wv, wo, freqs_cos, freqs_sin, mask, start_pos=0, **_):
    """GQA attention prefill on 8 TRN2 NeuronCores. Full inputs in, full output out."""
    import os
    global last_exec_time_ns
    from concourse.bass_utils import run_bass_kernel_spmd

    nc = _get_nc()
    in_maps = host_prep(x_norm, wq, wk, wv, wo, freqs_cos, freqs_sin, mask)
    trace = os.environ.get("BASS_KERNEL_TRACE", "0") == "1"
    res = run_bass_kernel_spmd(nc, in_maps, core_ids=list(range(8)), trace=trace)
    last_exec_time_ns = res.exec_time_ns
    global last_trace_path
    last_trace_path = (res.instructions_and_trace or (None, None))[1]
    return assemble(res.results)



# revision 36
# speedup vs baseline: 1.0477x; 1.0477x over previous
"""Bass/Tile SPMD kernel for GQA attention prefill (B=2,S=2048,D=4096,H=32,KVH=8,HD=128).

Sharding: 8 cores = 2 batch-groups x 4 cores. Core r of a batch group owns
four 128-row q-tiles, one per causal-need quartile: [12+r | 8+r | 4+r | r]
(SBUF column slots A..D). Total causal work is uniform across cores and the
SPMD instruction graph is identical; per-core causality comes from mask DATA.

Per core:
  phase 1: K^T projection (ch-outer so the first matmul starts ~1.3MB into
           the x DMA) + RoPE -> DRAM bounce -> AllGather over the 4-core
           batch group; V projection; Q projection + RoPE (first 6 ets'
           weights prefetched ahead of the wv stream; the V AllGather is
           deferred into the Q window to spread HBM demand; kh=0's K/V
           gather is prefetched on the GpSimd queue behind the collectives).
  phase 2: attention, [k,q] orientation. Step kt (0..15) covers cols [0:w],
           w = 512-128*(kt//4); the one slot whose diagonal can fall in kt
           is always cols [w-128:w], masked with per-core data. Scores are
           emitted 3 steps ahead of PV so the in-order PE never waits on the
           ScalarE exp. Rowsum: P-tile pairs pre-added on Vector, then a
           [128,128]-ones matmul (half the moving rows; result lands
           pre-broadcast on all 128 partitions), fast-approx reciprocal +
           multiply on Vector.
  phase 3: out rows = attn_T @ wo with the full wo streamed once.
           Output rows are disjoint across cores -> no reduction collective.

All data bf16 (1 PE cycle/row); PSUM accumulation f32. Measured PE rate is
~1.95GHz (board power throttle caps utilization at 0.8125 of 2.4GHz).
"""
import numpy as np
import concourse.bass as bass
import concourse.mybir as mybir
import concourse.tile as tile
from concourse import bacc

F32 = mybir.dt.float32
R32 = mybir.dt.float32r
BF16 = mybir.dt.bfloat16

B, S, D = 2, 2048, 4096
H, KVH, HD = 32, 8, 128
NBLK, BLK = 8, 256          # causal query blocks
SW = 512                    # stripe rows per core (2 blocks)
NDT = D // 128              # 32 contraction tiles
NET_Q = (H * HD) // 128     # 32
NET_KV = (KVH * HD) // 128  # 8
NKT0, NKT1 = 8, 16          # uniform k-tile counts (low block, high block)
SCALE = float(1.0 / np.sqrt(HD))
KV_REGION = 1024 * 512      # elems: K slab [1024,512] / V slab [512,1024]

REPLICA_GROUPS = [[0, 1, 2, 3], [4, 5, 6, 7]]


def _r(ap):
    return ap


def build():
    nc = bacc.Bacc("TRN2", target_bir_lowering=False, debug=False, num_devices=8)

    xt = nc.declare_dram_parameter("xt", [D, SW], BF16, isOutput=False)
    wqt = nc.declare_dram_parameter("wqt", [NET_Q, 4, 128, 8, 128], BF16, isOutput=False)
    wkt = nc.declare_dram_parameter("wkt", [NET_KV, 4, 128, 8, 128], BF16, isOutput=False)
    wvt = nc.declare_dram_parameter("wvt", [D, KVH * HD], BF16, isOutput=False)
    wot = nc.declare_dram_parameter("wot", [H * HD, D], BF16, isOutput=False)
    cos2 = nc.declare_dram_parameter("cos2", [128, SW], BF16, isOutput=False)
    sin2 = nc.declare_dram_parameter("sin2", [128, SW], BF16, isOutput=False)
    mq = nc.declare_dram_parameter("mq", [128, 16 * 128], BF16, isOutput=False)
    out = nc.declare_dram_parameter("out", [SW, D], F32, isOutput=True)

    with tile.TileContext(nc) as tc:
        _body(nc, tc, xt.ap(), wqt.ap(), wkt.ap(), wvt.ap(), wot.ap(),
              cos2.ap(), sin2.ap(), mq.ap(), out.ap())

    nc.compile()
    return nc


def _body(nc, tc, xt, wqt, wkt, wvt, wot, cos2, sin2, mq, out):
    from contextlib import ExitStack

    es = ExitStack()
    with es:
        # ---- persistent pools (live through phases 1+2) ----
        const_pool = es.enter_context(tc.tile_pool(name="consts", bufs=1))
        qkv_pool = es.enter_context(tc.tile_pool(name="qkv", bufs=1))
        dram = es.enter_context(tc.tile_pool(name="dram", bufs=1, space="DRAM"))

        cos_sb = const_pool.tile([128, SW], BF16)
        sin_sb = const_pool.tile([128, SW], BF16)
        mq_sb = const_pool.tile([128, 16, 128], BF16)
        ones_sb = const_pool.tile([128, 128], BF16)
        # consts ride the (otherwise idle) Scalar engine's DMA trigger queue
        # so the Sync queue starts on x/weight chunks immediately.
        nc.scalar.dma_start(cos_sb[:], cos2)
        nc.scalar.dma_start(sin_sb[:], sin2)
        nc.scalar.dma_start(mq_sb[:], mq.rearrange("p (t c) -> p t c", c=128))
        nc.vector.memset(ones_sb[:], 1.0)

        q_sb = qkv_pool.tile([128, NET_Q, SW], BF16)      # Q_T, head-major
        k_in = dram.tile([KV_REGION], BF16)
        v_in = dram.tile([KV_REGION], BF16)
        k_out = dram.tile([4, KV_REGION], BF16)
        v_out = dram.tile([4, KV_REGION], BF16)
        kreg = [k_out[gs].rearrange("(et p c) -> p et c", et=NET_KV, p=128)
                for gs in range(4)]
        vreg = [v_out[gs].rearrange("(st p e) -> p st e", st=4, p=128)
                for gs in range(4)]

        # ================= phase 1: projections =================
        with tc.tile_pool(name="p1x", bufs=1) as xpool, \
             tc.tile_pool(name="p1w", bufs=32) as wpool, \
             tc.tile_pool(name="p1wv", bufs=48) as wvpool, \
             tc.tile_pool(name="p1kv", bufs=1) as kvpool, \
             tc.tile_pool(name="p1rope", bufs=2) as rpool:

            x_sb = xpool.tile([128, NDT, SW], BF16)
            xtr = xt.rearrange("(dt p) c -> p dt c", p=128)

            k_sb = kvpool.tile([128, NET_KV, SW], BF16)
            v_sb = kvpool.tile([128, SW // 128, KVH * HD], BF16)

            def rope(ps, dst):
                raw = rpool.tile([128, SW], BF16, tag="rope_raw")
                sw_t = rpool.tile([128, SW], BF16, tag="rope_sw")
                nc.vector.tensor_copy(raw[:], ps)
                nc.sync.dma_start(sw_t[0:64, :], raw[64:128, :])
                nc.sync.dma_start(sw_t[64:128, :], raw[0:64, :])
                nc.vector.tensor_mul(dst, ps, cos_sb[:])
                nc.vector.tensor_mul(sw_t[:], sw_t[:], sin_sb[:])
                nc.vector.tensor_add(dst, dst, sw_t[:])

            # K projection + RoPE (first: feeds the AllGather). ch-outer
            # order: each ch needs only x chunk ch, so the first matmul
            # starts after ~1.3 MB of DMA instead of the full 4.2 MB x.
            with tc.tile_pool(name="p1kps", bufs=1, space="PSUM") as kpspool:
                kps = [kpspool.tile([128, SW], F32, tag=f"kps{et}",
                                    name=f"kps{et}")
                       for et in range(NET_KV)]
                for ch in range(4):
                    nc.sync.dma_start(
                        x_sb[:, ch * 8:(ch + 1) * 8],
                        xtr[:, ch * 8:(ch + 1) * 8])
                    for et in range(NET_KV):
                        w = wpool.tile([128, 8, 128], BF16, tag="wchunk")
                        nc.sync.dma_start(w[:], wkt[et, ch])
                        for dl in range(8):
                            dt = ch * 8 + dl
                            nc.tensor.matmul(
                                kps[et][:], _r(w[:, dl]), _r(x_sb[:, dt]),
                                start=(dt == 0), stop=(dt == NDT - 1))
                for et in range(NET_KV):
                    rope(kps[et][:], k_sb[:, et])
            nc.gpsimd.dma_start(
                k_in.rearrange("(et p c) -> p et c", et=NET_KV, p=128),
                k_sb[:])
            nc.gpsimd.collective_compute(
                "AllGather", mybir.AluOpType.bypass,
                replica_groups=REPLICA_GROUPS,
                ins=[k_in.opt()], outs=[k_out.opt()])

            with tc.tile_pool(name="p1ps", bufs=2, space="PSUM") as pspool, \
                 tc.tile_pool(name="p1psv", bufs=1, space="PSUM") as psvpool:

                def project(dst_slice, w_dram, et, pre=None):
                    # dst_slice: [128, SW] target; accumulate over 32 d-tiles
                    ps = pspool.tile([128, SW], F32, tag="proj_ps")
                    for ch in range(4):  # 8 d-tiles per weight chunk
                        if pre is not None:
                            w = pre[ch]
                        else:
                            w = wpool.tile([128, 8, 128], BF16, tag="wchunk")
                            nc.sync.dma_start(w[:], w_dram[et, ch])
                        for dl in range(8):
                            dt = ch * 8 + dl
                            nc.tensor.matmul(
                                ps[:], _r(w[:, dl]), _r(x_sb[:, dt]),
                                start=(dt == 0), stop=(dt == NDT - 1))
                    rope(ps[:], dst_slice)

                # V projection (natural [s, e] orientation; x as stationary)
                for ech in range(2):
                    vps = [psvpool.tile([128, 512], F32, tag=f"vps{st}",
                                        name=f"vps{st}")
                           for st in range(4)]
                    for dt in range(NDT):
                        wv = wvpool.tile([128, 512], BF16, tag="wvchunk")
                        nc.sync.dma_start(
                            wv[:],
                            wvt.rearrange("(dt p) e -> p dt e", p=128)[
                                :, dt, ech * 512:(ech + 1) * 512])
                        for st in range(4):
                            nc.tensor.matmul(
                                vps[st][:],
                                _r(x_sb[:, dt, st * 128:(st + 1) * 128]),
                                _r(wv[:]),
                                start=(dt == 0), stop=(dt == NDT - 1))
                    for st in range(4):
                        nc.vector.tensor_copy(
                            v_sb[:, st, ech * 512:(ech + 1) * 512], vps[st][:])
                nc.gpsimd.dma_start(
                    v_in.rearrange("(st p e) -> p st e", st=4, p=128),
                    v_sb[:])
                nc.gpsimd.collective_compute(
                    "AllGather", mybir.AluOpType.bypass,
                    replica_groups=REPLICA_GROUPS,
                    ins=[v_in.opt()], outs=[v_out.opt()])

                # prefetch the first 6 Q-weight ets behind the wv stream so
                # Q starts hot (V's PE time covers the queue delay). During
                # an AllGather the collective owns the DMA engines and the
                # weight streams starve: the deep wv/wq pools (fully-resident
                # wv before AG_K, ~8MB of wq in flight around AG_V) ride
                # those windows out.
                preq = []
                for et in range(6):
                    chunks = []
                    for ch in range(4):
                        w = wpool.tile([128, 8, 128], BF16, tag="wchunk")
                        nc.sync.dma_start(w[:], wqt[et, ch])
                        chunks.append(w)
                    preq.append(chunks)

                # Q projection + RoPE (overlaps the AllGathers)
                for et in range(NET_Q):
                    project(q_sb[:, et], wqt, et,
                            pre=preq[et] if et < len(preq) else None)

        # ================= phase 2: attention =================
        attn_pool = es.enter_context(tc.tile_pool(name="attn", bufs=1))
        attn_sb = attn_pool.tile([128, NET_Q, SW], BF16)  # attn_T, head-major

        with tc.tile_pool(name="p2kv", bufs=3) as kvload, \
             tc.tile_pool(name="p2p", bufs=6) as ppool, \
             tc.tile_pool(name="p2n", bufs=2) as npool, \
             tc.tile_pool(name="p2ps_s", bufs=4, space="PSUM") as ps_s, \
             tc.tile_pool(name="p2ps_o", bufs=2, space="PSUM") as ps_o, \
             tc.tile_pool(name="p2ps_r", bufs=2, space="PSUM") as ps_r:

            def load_kv(kh, dma_start):
                k_kh = kvload.tile([128, NKT1 * 128], BF16, tag="k_kh")
                v_kh = kvload.tile([128, NKT1, 128], BF16, tag="v_kh")
                for t in range(16):
                    gsrc = t % 4
                    slot = 3 - t // 4
                    dma_start(
                        k_kh[:, t * 128:(t + 1) * 128],
                        kreg[gsrc][:, kh, slot * 128:(slot + 1) * 128])
                    dma_start(
                        v_kh[:, t, :],
                        vreg[gsrc][:, slot, kh * 128:(kh + 1) * 128])
                return k_kh, v_kh

            # 4-slot exact-causal structure. Core r of each batch group owns
            # q-tiles (128 rows) [12+r | 8+r | 4+r | r] in cols [A|B|C|D].
            # Global k-tile t lives on group core t%4 at slot 3-t//4.
            # Step kt covers cols [0:w], w = 512-128*(kt//4); the slot whose
            # diagonal can fall in kt is always cols [w-128:w], masked with
            # per-core data (ones / triangle / zeros).
            for kh in range(KVH):
                # gather loads ride the GpSimd queue: it hosts the AllGather
                # ops, so in-order queue semantics make every load strictly
                # follow collective completion (Sync-queue loads would rely
                # on timing margin only — observed to flake).
                k_kh, v_kh = load_kv(kh, nc.gpsimd.dma_start)

                for qh4 in range(4):
                    h = kh * 4 + qh4
                    ops = ps_o.tile([128, 2 * BLK], F32, tag="ops")
                    rps = ps_r.tile([128, 2 * BLK], F32, tag="rps")

                    NSTEP = 16
                    sps_t = [None] * NSTEP
                    p_t = [None] * NSTEP
                    WID = [512 - 128 * (kt // 4) for kt in range(NSTEP)]

                    def emit_scores(i):
                        w = WID[i]
                        sps = ps_s.tile([128, 2 * BLK], F32, tag="sps")
                        sps_t[i] = sps
                        nc.tensor.matmul(
                            sps[:, 0:w], k_kh[:, i * 128:(i + 1) * 128],
                            q_sb[:, h, 0:w], start=True, stop=True)

                    def emit_expmask(i):
                        w = WID[i]
                        p_sb = ppool.tile([128, 2 * BLK], BF16, tag="p_sb")
                        p_t[i] = p_sb
                        nc.scalar.activation(
                            p_sb[:, 0:w], sps_t[i][:, 0:w],
                            mybir.ActivationFunctionType.Exp, scale=SCALE)
                        nc.vector.tensor_mul(
                            p_sb[:, w - 128:w], p_sb[:, w - 128:w],
                            mq_sb[:, i])

                    def emit_pv(i):
                        w = WID[i]
                        nc.tensor.matmul(
                            ops[:, 0:w], v_kh[:, i], p_t[i][:, 0:w],
                            start=(i == 0), stop=(i == NSTEP - 1),
                            skip_group_check=True)

                    pair_t = [None] * 8

                    def emit_rs_add(j):
                        # rowsum for step pair (2j, 2j+1): pre-add the two P
                        # tiles on Vector (same width within a quartile) so
                        # the ones-matmul runs half the moving rows.
                        w = WID[2 * j]
                        p_pair = ppool.tile([128, 2 * BLK], BF16,
                                            tag="p_pair")
                        pair_t[j] = p_pair
                        nc.vector.tensor_add(
                            p_pair[:, 0:w], p_t[2 * j][:, 0:w],
                            p_t[2 * j + 1][:, 0:w])

                    def emit_rs_mm(j):
                        w = WID[2 * j]
                        nc.tensor.matmul(
                            rps[:, 0:w], ones_sb[:], pair_t[j][:, 0:w],
                            start=(j == 0), stop=(j == 7),
                            skip_group_check=True)

                    emit_scores(0)
                    emit_scores(1)
                    emit_scores(2)
                    for i in range(NSTEP):
                        emit_expmask(i)
                        if i % 2 == 1:
                            emit_rs_add(i // 2)
                        if i + 3 < NSTEP:
                            emit_scores(i + 3)
                        emit_pv(i)
                        if i >= 2 and i % 2 == 0:
                            emit_rs_mm(i // 2 - 1)
                    emit_rs_mm(7)

                    # normalize: rps rows are all identical (ones stationary
                    # is [128,128]) so no partition broadcast is needed.
                    rcpb = npool.tile([128, 2 * BLK], F32, tag="rcpb")
                    nc.vector.reciprocal_approx_fast(rcpb[:], rps[:])
                    nc.vector.tensor_mul(attn_sb[:, h, :], ops[:], rcpb[:])

        # ================= phase 3: output projection =================
        with tc.tile_pool(name="p3w", bufs=16) as wopool, \
             tc.tile_pool(name="p3y", bufs=4) as ypool, \
             tc.tile_pool(name="p3ps", bufs=1, space="PSUM") as ps_y:

            wotr = wot.rearrange("(et p) d -> p et d", p=128)
            for dch in range(8):
                yps = [ps_y.tile([128, 512], F32, tag=f"yps{st}", name=f"yps{st}")
                       for st in range(4)]
                for et in range(NET_Q):
                    w = wopool.tile([128, 512], BF16, tag="wo_chunk")
                    nc.sync.dma_start(
                        w[:], wotr[:, et, dch * 512:(dch + 1) * 512])
                    for st in range(4):
                        nc.tensor.matmul(
                            yps[st][:],
                            _r(attn_sb[:, et, st * 128:(st + 1) * 128]),
                            _r(w[:]),
                            start=(et == 0), stop=(et == NET_Q - 1))
                for st in range(4):
                    y = ypool.tile([128, 512], F32, tag="y_sb")
                    nc.vector.tensor_copy(y[:], yps[st][:])
                    nc.sync.dma_start(
                        out[st * 128:(st + 1) * 128, dch * 512:(dch + 1) * 512],
                        y[:])


# ======================= host side =======================

def _perm_idx(nheads):
    """Within each 128-dim head block: evens then odds."""
    idx = []
    for hh in range(nheads):
        base = hh * HD
        idx.extend(base + j for j in range(0, HD, 2))
        idx.extend(base + j for j in range(1, HD, 2))
    return np.array(idx)


def host_prep(x_norm, wq, wk, wv, wo, freqs_cos, freqs_sin, mask):
    """Build the 8 per-core input maps."""
    import ml_dtypes
    bf16 = ml_dtypes.bfloat16
    f32 = np.float32
    x_norm = np.ascontiguousarray(x_norm, dtype=f32)
    wq = np.asarray(wq, dtype=f32)
    wk = np.asarray(wk, dtype=f32)
    wv = np.asarray(wv, dtype=f32)
    wo = np.asarray(wo, dtype=f32)
    fc = np.asarray(freqs_cos, dtype=f32)
    fs = np.asarray(freqs_sin, dtype=f32)
    mask = np.asarray(mask, dtype=f32)

    pq = _perm_idx(H)
    pk = _perm_idx(KVH)
    def pack_w(wt):
        # wt: [D, E] (d-major) -> [E//128, 4, 128, 8, 128] matching SBUF chunks
        E = wt.shape[1]
        p_ = wt.reshape(4, 8, 128, E // 128, 128)     # [ch, dl, p, et, e]
        return np.ascontiguousarray(p_.transpose(3, 0, 2, 1, 4))

    wqt = pack_w(wq[pq, :].T)                        # [32, 4, 128, 8, 128]
    wkt = pack_w(wk[pk, :].T)                        # [8, 4, 128, 8, 128]
    wvt = np.ascontiguousarray(wv.T)                 # [D, 1024]
    wot = np.ascontiguousarray(wo.T)                 # [4096, D]

    cosT = fc.T                                      # [64, S]
    sinT = fs.T
    cos_full = np.concatenate([cosT, cosT], axis=0)  # [128, S]
    sin_full = np.concatenate([-sinT, sinT], axis=0)

    keep = (mask == 0.0)                             # [S, S] bool, k<=q

    in_maps = []
    for c in range(8):
        b, r = c // 4, c % 4
        tiles = [12 + r, 8 + r, 4 + r, r]            # slots A..D (128 q each)
        cols = np.concatenate(
            [np.arange(t * 128, (t + 1) * 128) for t in tiles])
        xt = np.ascontiguousarray(x_norm[b].T[:, cols])
        cos2 = np.ascontiguousarray(cos_full[:, cols])
        sin2 = np.ascontiguousarray(sin_full[:, cols])

        # mq[:, kt, :]: mask for the one slot whose diagonal can fall in
        # step kt — q-tile 4*(kt//4)+r at cols [w-128:w].
        m = np.zeros((128, 16, 128), dtype=f32)
        for kt in range(16):
            t_masked = 4 * (kt // 4) + r
            qrows = np.arange(t_masked * 128, (t_masked + 1) * 128)
            krows = np.arange(kt * 128, (kt + 1) * 128)
            m[:, kt, :] = keep[np.ix_(qrows, krows)].T

        in_maps.append({
            "xt": xt.astype(bf16),
            "wqt": wqt.astype(bf16), "wkt": wkt.astype(bf16),
            "wvt": wvt.astype(bf16), "wot": wot.astype(bf16),
            "cos2": cos2.astype(bf16), "sin2": sin2.astype(bf16),
            "mq": np.ascontiguousarray(m.reshape(128, 16 * 128)).astype(bf16),
        })
    return in_maps


def assemble(results):
    """results: list of 8 dicts with 'out' [512, 4096] -> full [B, S, D]."""
    full = np.empty((B, S, D), dtype=np.float32)
    for c in range(8):
        b, r = c // 4, c % 4
        tiles = [12 + r, 8 + r, 4 + r, r]
        rc = results[c]["out"]
        for slot, t in enumerate(tiles):
            full[b, t * 128:(t + 1) * 128] = rc[slot * 128:(slot + 1) * 128]
    return full


# ======================= public entry point =======================

_NC_CACHE = {}
last_exec_time_ns = None


def _get_nc():
    if "nc" not in _NC_CACHE:
        _NC_CACHE["nc"] = build()
    return _NC_CACHE["nc"]


def kernel(x_norm, wq, wk, wv, wo, freqs_cos, freqs_sin, mask, start_pos=0, **_):
    """GQA attention prefill on 8 TRN2 NeuronCores. Full inputs in, full output out."""
    import os
    global last_exec_time_ns
    from concourse.bass_utils import run_bass_kernel_spmd

    nc = _get_nc()
    in_maps = host_prep(x_norm, wq, wk, wv, wo, freqs_cos, freqs_sin, mask)
    trace = os.environ.get("BASS_KERNEL_TRACE", "0") == "1"
    res = run_bass_kernel_spmd(nc, in_maps, core_ids=list(range(8)), trace=trace)
    last_exec_time_ns = res.exec_time_ns
    global last_trace_path
    last_trace_path = (res.instructions_and_trace or (None, None))[1]
    return assemble(res.results)

